# revision 17
# baseline (speedup 1.0000x reference)
"""Trainium2 Bass kernel for nn_MistralAudioCodebook (VQ codebook).

Strategy (8 NeuronCores, data-parallel over batch; 2 examples/core):
  Pass A (NEFF-A): coarse full cdist scores S = x.e_k - 0.5||e_k||^2 via
    float32r matmuls (full-rate fp32-width PE mode, ~tf32 precision),
    per-vector top-8 via DVE InstMax/InstMaxIndex on fp16-scaled scores.
    Also computes the whole acoustic FSQ path.
  Host: per-example union of top-8 candidate codes (~210 of 2048), padded
    to 256; exact fp32 candidate table built on host.
  Pass B (NEFF-B): exact fp32 cdist against the 256 candidates, argmax,
    gather of winning embedding rows (indirect DMA) + PE transpose to
    d-major, producing quantized semantic rows and codes.

Empirically (and by construction) the true fp32 argmin is always within
the coarse top-2, so the top-8 union contains it with huge margin.
"""

import numpy as np

import concourse.bass as bass
import concourse.mybir as mybir
import concourse.tile as tile
from concourse import bacc
from concourse.bass_utils import run_bass_kernel_spmd
from concourse.masks import make_identity

F32 = mybir.dt.float32
F32R = mybir.dt.float32r
F16 = mybir.dt.float16
U32 = mybir.dt.uint32
I32 = mybir.dt.int32

SEM_DIM = 512
KCB = 2048          # codebook size
ACO = 32
B, T = 16, 4096
NCORES = 8
BPC = B // NCORES   # examples per core (2)
NT = BPC * T // 128  # t-tiles per core (64)
TPB = T // 128       # t-tiles per example (32)
C = 256              # candidate set size per example
EPS = 1e-5
HALF = 4.0
RND = 12582912.0     # 1.5 * 2^23 : RNE round-to-int magic constant

_TRACE = {"on": False, "kwargs": {}}
_LAST = {}


def _r(ap):
    return ap.bitcast(F32R)


def build_neff_a(KP=KCB):
    nc = bacc.Bacc("TRN2", target_bir_lowering=False, debug=False)
    NKT = KP // 128    # k-tiles in emb prep
    NKC = KP // 512    # 512-wide psum chunks

    xs = nc.dram_tensor("xs", [BPC, SEM_DIM, T], F32R, kind="ExternalInput")
    xa = nc.dram_tensor("xa", [BPC * ACO, T], F32, kind="ExternalInput")
    es = nc.dram_tensor("es", [KP, SEM_DIM], F32, kind="ExternalInput")
    cu = nc.dram_tensor("cu", [KP, 1], F32, kind="ExternalInput")
    nz = nc.dram_tensor("nz", [BPC * ACO, T], F32, kind="ExternalInput")
    masks = nc.dram_tensor("masks", [BPC * ACO, 3], F32, kind="ExternalInput")

    top8i = nc.dram_tensor("top8i", [128, NT * 8], U32, kind="ExternalOutput")
    q_aco = nc.dram_tensor("q_aco", [BPC * ACO, T], F32, kind="ExternalOutput")
    c_aco = nc.dram_tensor("c_aco", [BPC * ACO, T], I32, kind="ExternalOutput")

    with tile.TileContext(nc) as tc:
        with tc.tile_pool(name="const", bufs=1) as cpool:
            # Persistent tensors
            embT = [cpool.tile([128, KP], F32R, tag=f"embT{dc}",
                               name=f"embT{dc}") for dc in range(4)]
            negnorm = cpool.tile([1, KP], F32R, tag="negnorm")
            ones_col = cpool.tile([128, 1], F32R, tag="ones_col")   # lhsT for col-sums
            ones_row = cpool.tile([1, 128], F32R, tag="ones_row")   # lhsT for bias bcast
            ident = cpool.tile([128, 128], F32, tag="ident")
            ones_f = cpool.tile([128, 1], F32, tag="ones_f")
            onesr_f = cpool.tile([1, 128], F32, tag="onesr_f")
            nc.vector.memset(ones_f[:], 1.0)
            nc.vector.memset(onesr_f[:], 1.0)
            nc.scalar.copy(out=ones_col[:], in_=ones_f[:])
            nc.scalar.copy(out=ones_row[:], in_=onesr_f[:])
            make_identity(nc, ident[:])

            # ---- embedding prep: emb = es / clip(cu, EPS); embT = emb.T ----
            with (
                tc.tile_pool(name="prep", bufs=2) as ppool,
                tc.tile_pool(name="prep_ps", bufs=2, space="PSUM") as pppool,
            ):
                for kt in range(NKT):
                    cu_t = ppool.tile([128, 1], F32, tag="cu")
                    nc.sync.dma_start(out=cu_t[:], in_=cu[kt * 128:(kt + 1) * 128, :])
                    inv = ppool.tile([128, 1], F32, tag="inv")
                    nc.vector.tensor_scalar_max(cu_t[:], cu_t[:], EPS)
                    nc.vector.reciprocal(inv[:], cu_t[:])
                    es_t = ppool.tile([128, SEM_DIM], F32, tag="es")
                    nc.sync.dma_start(out=es_t[:], in_=es[kt * 128:(kt + 1) * 128, :])
                    emb_t = ppool.tile([128, SEM_DIM], F32, tag="emb")
                    nc.vector.tensor_scalar_mul(emb_t[:], es_t[:], inv[:, 0:1])
                    for dc in range(4):
                        tp = pppool.tile([128, 128], F32, tag="tp")
                        nc.tensor.transpose(tp[:], emb_t[:, dc * 128:(dc + 1) * 128],
                                            ident[:])
                        nc.scalar.copy(
                            out=embT[dc][:, kt * 128:(kt + 1) * 128], in_=tp[:])

                # ---- negnorm[k] = -0.5 * sum_d embT[d,k]^2  (via ones matmul) ----
                npsums = [pppool.tile([1, 512], F32, tag=f"np{kc}",
                                      name=f"np{kc}", bufs=1) for kc in range(NKC)]
                for kc in range(NKC):
                    for dc in range(4):
                        sq = ppool.tile([128, 512], F32R, tag="sq")
                        sl = embT[dc][:, kc * 512:(kc + 1) * 512]
                        nc.gpsimd.tensor_mul(sq[:], sl, sl)
                        nc.tensor.matmul(npsums[kc][:], _r(ones_col[:]), _r(sq[:]),
                                         start=(dc == 0), stop=(dc == 3))
                    nc.scalar.activation(negnorm[0:1, kc * 512:(kc + 1) * 512],
                                         npsums[kc][:],
                                         mybir.ActivationFunctionType.Copy,
                                         scale=-0.5)

            # ---- main cdist loop (with interleaved acoustic chunks) ----
            iacc = cpool.tile([128, NT * 8], U32, tag="iacc")
            NCHUNK = 8
            W = T // NCHUNK

            with (
                tc.tile_pool(name="xp", bufs=4) as xpool,
                tc.tile_pool(name="sp", bufs=3) as spool,
                tc.tile_pool(name="mp", bufs=4) as mpool,
                tc.tile_pool(name="aco", bufs=2) as apool,
                tc.tile_pool(name="ps", bufs=8, space="PSUM") as pspool,
            ):
                msk = cpool.tile([BPC * ACO, 3], F32, tag="msk")
                nc.sync.dma_start(out=msk[:], in_=masks[:, :])

                def aco_chunk(ch):
                    csl = slice(ch * W, (ch + 1) * W)
                    xa_t = apool.tile([BPC * ACO, W], F32, tag="xa", name="xa_t")
                    nc.sync.dma_start(out=xa_t[:], in_=xa[:, csl])
                    zb = apool.tile([BPC * ACO, W], F32, tag="zb", name="zb")
                    nc.scalar.activation(zb[:], xa_t[:],
                                         mybir.ActivationFunctionType.Tanh)
                    nc.gpsimd.tensor_scalar_mul(zb[:], zb[:], HALF)
                    zq = apool.tile([BPC * ACO, W], F32, tag="zq", name="zq")
                    nc.gpsimd.tensor_scalar_add(zq[:], zb[:], RND)
                    nc.gpsimd.tensor_scalar_sub(zq[:], zq[:], RND)
                    ns = apool.tile([BPC * ACO, W], F32, tag="ns", name="ns")
                    nc.sync.dma_start(out=ns[:], in_=nz[:, csl])
                    nc.gpsimd.tensor_scalar(ns[:], ns[:], 2.0 / 9.0 * HALF,
                                            HALF / 9.0, mybir.AluOpType.mult,
                                            mybir.AluOpType.subtract)
                    zd = apool.tile([BPC * ACO, W], F32, tag="zd", name="zd")
                    nc.gpsimd.tensor_add(zd[:], zb[:], ns[:])
                    nc.gpsimd.tensor_scalar(zd[:], zd[:], -HALF, HALF,
                                            mybir.AluOpType.max,
                                            mybir.AluOpType.min)
                    zo = apool.tile([BPC * ACO, W], F32, tag="zo", name="zo")
                    nc.gpsimd.tensor_scalar_mul(zo[:], zq[:], msk[:, 0:1])
                    nc.gpsimd.tensor_scalar_mul(zd[:], zd[:], msk[:, 1:2])
                    nc.gpsimd.tensor_add(zo[:], zo[:], zd[:])
                    nc.gpsimd.tensor_scalar_mul(zb[:], zb[:], msk[:, 2:3])
                    nc.gpsimd.tensor_add(zo[:], zo[:], zb[:])
                    cf = apool.tile([BPC * ACO, W], F32, tag="cf", name="cf")
                    nc.gpsimd.tensor_scalar_add(cf[:], zo[:], HALF + RND)
                    nc.gpsimd.tensor_scalar_sub(cf[:], cf[:], RND)
                    ci = apool.tile([BPC * ACO, W], I32, tag="ci", name="ci")
                    nc.gpsimd.tensor_copy(ci[:], cf[:])
                    nc.sync.dma_start(out=c_aco[:, csl], in_=ci[:])
                    nc.gpsimd.tensor_scalar_mul(zo[:], zo[:], 1.0 / HALF)
                    nc.sync.dma_start(out=q_aco[:, csl], in_=zo[:])

                for tt in range(NT):
                    if tt % (NT // NCHUNK) == 2:
                        aco_chunk(tt // (NT // NCHUNK))
                    b = tt // TPB
                    t0 = (tt % TPB) * 128
                    xt = xpool.tile([128, SEM_DIM], F32R, tag="xt")
                    nc.sync.dma_start(
                        out=xt[:].rearrange("p (dc t) -> p dc t", t=128),
                        in_=xs[b, :, t0:t0 + 128].rearrange(
                            "(dc p) t -> p dc t", p=128))
                    xts = [xt[:, dc * 128:(dc + 1) * 128] for dc in range(4)]
                    s16 = spool.tile([128, KP], F16, tag="s16")
                    for kc in range(NKC):
                        ps = pspool.tile([128, 512], F32, tag="ps")
                        ksl = slice(kc * 512, (kc + 1) * 512)
                        nc.tensor.matmul(ps[:], _r(ones_row[:]),
                                         _r(negnorm[0:1, ksl]),
                                         start=True, stop=False)
                        for dc in range(4):
                            nc.tensor.matmul(ps[:], xts[dc],
                                             _r(embT[dc][:, ksl]),
                                             start=False, stop=(dc == 3))
                        nc.scalar.activation(s16[:, ksl], ps[:],
                                             mybir.ActivationFunctionType.Copy,
                                             scale=1.0 / 16.0)
                    m8 = mpool.tile([128, 8], F16, tag="m8")
                    nc.vector.max(out=m8[:], in_=s16[:])
                    nc.vector.max_index(out=iacc[:, tt * 8:(tt + 1) * 8],
                                        in_max=m8[:], in_values=s16[:])
                nc.sync.dma_start(out=top8i[:, :], in_=iacc[:])

    nc.compile()
    return nc


def build_neff_b():
    nc = bacc.Bacc("TRN2", target_bir_lowering=False, debug=False)

    xs = nc.dram_tensor("xs", [BPC, SEM_DIM, T], F32, kind="ExternalInput")
    embc = [
        nc.dram_tensor(f"embc{b}", [C, SEM_DIM], F32, kind="ExternalInput")
        for b in range(BPC)
    ]
    negnc = nc.dram_tensor("negnc", [BPC, C], F32, kind="ExternalInput")

    idxb = nc.dram_tensor("idxb", [128, NT * 8], U32, kind="ExternalOutput")
    q_sem = nc.dram_tensor("q_sem", [BPC, SEM_DIM, T], F32, kind="ExternalOutput")

    with tile.TileContext(nc) as tc:
        with tc.tile_pool(name="const", bufs=1) as cpool:
            ident = cpool.tile([128, 128], F32, tag="ident")
            make_identity(nc, ident[:])
            embCT = [[cpool.tile([128, C], F32, tag=f"eCT{b}_{dc}",
                                 name=f"eCT{b}_{dc}")
                      for dc in range(4)] for b in range(BPC)]
            bias = [cpool.tile([128, C], F32, tag=f"bias{b}",
                               name=f"bias{b}") for b in range(BPC)]

            with (
                tc.tile_pool(name="prep", bufs=2) as ppool,
                tc.tile_pool(name="prep_ps", bufs=2, space="PSUM") as pppool,
            ):
                for b in range(BPC):
                    nc.sync.dma_start(
                        out=bias[b][:],
                        in_=negnc[b:b + 1, :].to_broadcast([128, C]))
                    for kt in range(C // 128):
                        ec = ppool.tile([128, SEM_DIM], F32, tag="ec")
                        nc.sync.dma_start(
                            out=ec[:], in_=embc[b][kt * 128:(kt + 1) * 128, :])
                        for dc in range(4):
                            tp = pppool.tile([128, 128], F32, tag="tp")
                            nc.tensor.transpose(
                                tp[:], ec[:, dc * 128:(dc + 1) * 128], ident[:])
                            nc.scalar.copy(
                                out=embCT[b][dc][:, kt * 128:(kt + 1) * 128],
                                in_=tp[:])

            iacc = cpool.tile([128, NT * 8], U32, tag="iacc")
            with (
                tc.tile_pool(name="xp", bufs=3) as xpool,
                tc.tile_pool(name="sc", bufs=2) as scpool,
                tc.tile_pool(name="mp", bufs=2) as mpool,
                tc.tile_pool(name="gp", bufs=2) as gpool,
                tc.tile_pool(name="ps", bufs=4, space="PSUM") as pspool,
                tc.tile_pool(name="tps", bufs=4, space="PSUM") as tpspool,
            ):
                for tt in range(NT):
                    b = tt // TPB
                    t0 = (tt % TPB) * 128
                    xt = xpool.tile([128, SEM_DIM], F32, tag="xt")
                    nc.sync.dma_start(
                        out=xt[:].rearrange("p (dc t) -> p dc t", t=128),
                        in_=xs[b, :, t0:t0 + 128].rearrange(
                            "(dc p) t -> p dc t", p=128))
                    ps = pspool.tile([128, C], F32, tag="ps")
                    for dc in range(4):
                        nc.tensor.matmul(ps[:], xt[:, dc * 128:(dc + 1) * 128],
                                         embCT[b][dc][:],
                                         start=(dc == 0), stop=(dc == 3))
                    sc = scpool.tile([128, C], F32, tag="sc")
                    nc.vector.tensor_add(sc[:], ps[:], bias[b][:])
                    m8 = mpool.tile([128, 8], F32, tag="m8")
                    i8 = iacc[:, tt * 8:(tt + 1) * 8]
                    nc.vector.max(out=m8[:], in_=sc[:])
                    nc.vector.max_index(out=i8, in_max=m8[:], in_values=sc[:])
                    g = gpool.tile([128, SEM_DIM], F32, tag="g")
                    nc.gpsimd.indirect_dma_start(
                        out=g[:], out_offset=None, in_=embc[b][:],
                        in_offset=bass.IndirectOffsetOnAxis(
                            ap=iacc[:, tt * 8:tt * 8 + 1], axis=0))
                    gt = gpool.tile([128, SEM_DIM], F32, tag="gt")
                    for dc in range(4):
                        tp = tpspool.tile([128, 128], F32, tag="tp")
                        nc.tensor.transpose(
                            tp[:], g[:, dc * 128:(dc + 1) * 128], ident[:])
                        nc.scalar.copy(out=gt[:, dc * 128:(dc + 1) * 128], in_=tp[:])
                    nc.sync.dma_start(
                        out=q_sem[b, :, t0:t0 + 128].rearrange(
                            "(dc p) t -> p dc t", p=128),
                        in_=gt[:].rearrange("p (dc t) -> p dc t", t=128))
                nc.sync.dma_start(out=idxb[:, :], in_=iacc[:])

    nc.compile()
    return nc


def kernel(x, embedding_sum, cluster_usage, noise, probs_sem, probs_aco):
    x = np.ascontiguousarray(np.asarray(x, dtype=np.float32))
    es = np.ascontiguousarray(np.asarray(embedding_sum, dtype=np.float32))
    cu = np.asarray(cluster_usage, dtype=np.float32)
    noise = np.asarray(noise, dtype=np.float32)
    probs_sem = np.asarray(probs_sem, dtype=np.float32)
    probs_aco = np.asarray(probs_aco, dtype=np.float32)

    # ---------- host: provably-safe codebook pruning for pass A ----------
    # Code k can appear in some vector's true top-8 only if
    # U_k = X*r_k - r_k^2/2 >= L8 = 8th-largest of (-X*r_k - r_k^2/2),
    # where X = max_t ||x_t||. Everything in fp64 with slack.
    emb64h = es.astype(np.float64) / np.clip(
        cu.astype(np.float64), EPS, None)[:, None]
    r64 = np.linalg.norm(emb64h, axis=1)
    X = float(np.sqrt((x[:, :SEM_DIM, :].astype(np.float64) ** 2)
                      .sum(1)).max())
    U = X * r64 - 0.5 * r64 * r64
    L8 = np.sort(-X * r64 - 0.5 * r64 * r64)[-8]
    surv = np.nonzero(U >= L8 - 1.0)[0]
    KP = min(KCB, max(512, int(np.ceil(len(surv) / 512.0)) * 512))
    surv_p = np.concatenate([surv, np.full(KP - len(surv), surv[0])])[:KP]
    surv_p = surv_p.astype(np.int64)

    # ---------- pass A ----------
    nc_a = build_neff_a(KP)
    in_maps_a = []
    es_p = np.ascontiguousarray(es[surv_p])
    cu_p = np.ascontiguousarray(cu[surv_p].reshape(KP, 1))
    for c in range(NCORES):
        gb = slice(c * BPC, (c + 1) * BPC)
        xs = np.ascontiguousarray(x[gb, :SEM_DIM, :])
        xa = np.ascontiguousarray(x[gb, SEM_DIM:, :].reshape(BPC * ACO, T))
        nz = np.ascontiguousarray(noise[gb].reshape(BPC * ACO, T))
        masks = np.zeros((BPC * ACO, 3), dtype=np.float32)
        for b in range(BPC):
            p = probs_aco[c * BPC + b]
            col = 0 if p < 0.5 else (1 if p < 0.75 else 2)
            masks[b * ACO:(b + 1) * ACO, col] = 1.0
        in_maps_a.append({"xs": xs, "xa": xa, "es": es_p, "cu": cu_p,
                          "nz": nz, "masks": masks})
    res_a = run_bass_kernel_spmd(nc_a, in_maps_a, list(range(NCORES)),
                                 trace=_TRACE["on"], **_TRACE["kwargs"])
    _LAST["a"] = res_a
    _LAST["nc_a"] = nc_a

    # ---------- host: candidate unions + exact tables ----------
    emb64 = es.astype(np.float64) / np.clip(
        cu.astype(np.float64), EPS, None)[:, None]
    emb32 = emb64.astype(np.float32)
    nrm64 = -0.5 * (emb32.astype(np.float64) ** 2).sum(1)

    c_lists = np.zeros((B, C), dtype=np.int64)
    for gb in range(B):
        c, b = gb // BPC, gb % BPC
        # top8i layout: [128, NT*8] -> vector (tt, p) top-8 at [p, tt*8:(tt+1)*8]
        ti = res_a.results[c]["top8i"].reshape(128, NT, 8)[:, b * TPB:(b + 1) * TPB]
        ti = surv_p[ti.reshape(-1).astype(np.int64)]
        uniq, counts = np.unique(ti, return_counts=True)
        if len(uniq) > C:
            keep = np.sort(uniq[np.argsort(-counts)[:C]])
        else:
            keep = uniq
        cl = np.sort(keep.astype(np.int64))
        c_lists[gb, :len(cl)] = cl
        c_lists[gb, len(cl):] = cl[-1] if len(cl) else 0

    # ---------- pass B ----------
    nc_b = build_neff_b()
    in_maps_b = []
    for c in range(NCORES):
        gb = slice(c * BPC, (c + 1) * BPC)
        xs = np.ascontiguousarray(x[gb, :SEM_DIM, :])
        m = {"xs": xs}
        negnc = np.zeros((BPC, C), dtype=np.float32)
        for b in range(BPC):
            cl = c_lists[c * BPC + b]
            m[f"embc{b}"] = np.ascontiguousarray(emb32[cl])
            negnc[b] = nrm64[cl].astype(np.float32)
        m["negnc"] = negnc
        in_maps_b.append(m)
    res_b = run_bass_kernel_spmd(nc_b, in_maps_b, list(range(NCORES)),
                                 trace=_TRACE["on"], **_TRACE["kwargs"])
    _LAST["b"] = res_b
    _LAST["nc_b"] = nc_b

    # ---------- assemble ----------
    quantized = np.empty((B, SEM_DIM + ACO, T), dtype=np.float32)
    codes = np.empty((B, 1 + ACO, T), dtype=np.int32)
    for c in range(NCORES):
        ra, rb = res_a.results[c], res_b.results[c]
        for b in range(BPC):
            gb = c * BPC + b
            if probs_sem[gb] < 0.5:
                quantized[gb, :SEM_DIM] = rb["q_sem"][b]
            else:
                quantized[gb, :SEM_DIM] = x[gb, :SEM_DIM]
            quantized[gb, SEM_DIM:] = ra["q_aco"][b * ACO:(b + 1) * ACO]
            j = rb["idxb"].reshape(128, NT, 8)[:, b * TPB:(b + 1) * TPB, 0]
            j = j.T.reshape(T)
            codes[gb, 0] = c_lists[gb][j.astype(np.int64)].astype(np.int32)
            codes[gb, 1:] = ra["c_aco"][b * ACO:(b + 1) * ACO]
    return quantized, codes


# revision 18
# speedup vs baseline: 1.0215x; 1.0215x over previous
"""Trainium2 Bass kernel for nn_MistralAudioCodebook (VQ codebook).

Strategy (8 NeuronCores, data-parallel over batch; 2 examples/core):
  Pass A (NEFF-A): coarse full cdist scores S = x.e_k - 0.5||e_k||^2 via
    float32r matmuls (full-rate fp32-width PE mode, ~tf32 precision),
    per-vector top-8 via DVE InstMax/InstMaxIndex on fp16-scaled scores.
    Also computes the whole acoustic FSQ path.
  Host: per-example union of top-8 candidate codes (~210 of 2048), padded
    to 256; exact fp32 candidate table built on host.
  Pass B (NEFF-B): exact fp32 cdist against the 256 candidates, argmax,
    gather of winning embedding rows (indirect DMA) + PE transpose to
    d-major, producing quantized semantic rows and codes.

Empirically (and by construction) the true fp32 argmin is always within
the coarse top-2, so the top-8 union contains it with huge margin.
"""

import numpy as np

import concourse.bass as bass
import concourse.mybir as mybir
import concourse.tile as tile
from concourse import bacc
from concourse.bass_utils import run_bass_kernel_spmd
from concourse.masks import make_identity

F32 = mybir.dt.float32
F32R = mybir.dt.float32r
F16 = mybir.dt.float16
U32 = mybir.dt.uint32
I32 = mybir.dt.int32

SEM_DIM = 512
KCB = 2048          # codebook size
ACO = 32
B, T = 16, 4096
NCORES = 8
BPC = B // NCORES   # examples per core (2)
NT = BPC * T // 128  # t-tiles per core (64)
TPB = T // 128       # t-tiles per example (32)
C = 256              # candidate set size per example
EPS = 1e-5
HALF = 4.0
RND = 12582912.0     # 1.5 * 2^23 : RNE round-to-int magic constant

_TRACE = {"on": False, "kwargs": {}}
_LAST = {}


def _r(ap):
    return ap.bitcast(F32R)


def build_neff_a(KP=KCB):
    nc = bacc.Bacc("TRN2", target_bir_lowering=False, debug=False)
    NKT = KP // 128    # k-tiles in emb prep
    NKC = KP // 512    # 512-wide psum chunks

    xs = nc.dram_tensor("xs", [BPC, SEM_DIM, T], F32R, kind="ExternalInput")
    xa = nc.dram_tensor("xa", [BPC * ACO, T], F32, kind="ExternalInput")
    es = nc.dram_tensor("es", [KP, SEM_DIM], F32, kind="ExternalInput")
    cu = nc.dram_tensor("cu", [KP, 1], F32, kind="ExternalInput")
    nz = nc.dram_tensor("nz", [BPC * ACO, T], F32, kind="ExternalInput")
    masks = nc.dram_tensor("masks", [BPC * ACO, 3], F32, kind="ExternalInput")

    top8i = nc.dram_tensor("top8i", [128, NT * 8], U32, kind="ExternalOutput")
    q_aco = nc.dram_tensor("q_aco", [BPC * ACO, T], F32, kind="ExternalOutput")
    c_aco = nc.dram_tensor("c_aco", [BPC * ACO, T], I32, kind="ExternalOutput")

    with tile.TileContext(nc) as tc:
        with tc.tile_pool(name="const", bufs=1) as cpool:
            # Persistent tensors
            embT = [cpool.tile([128, KP], F32R, tag=f"embT{dc}",
                               name=f"embT{dc}") for dc in range(4)]
            negnorm = cpool.tile([1, KP], F32R, tag="negnorm")
            ones_col = cpool.tile([128, 1], F32R, tag="ones_col")   # lhsT for col-sums
            ones_row = cpool.tile([1, 128], F32R, tag="ones_row")   # lhsT for bias bcast
            ident = cpool.tile([128, 128], F32, tag="ident")
            ones_f = cpool.tile([128, 1], F32, tag="ones_f")
            onesr_f = cpool.tile([1, 128], F32, tag="onesr_f")
            nc.vector.memset(ones_f[:], 1.0)
            nc.vector.memset(onesr_f[:], 1.0)
            nc.scalar.copy(out=ones_col[:], in_=ones_f[:])
            nc.scalar.copy(out=ones_row[:], in_=onesr_f[:])
            make_identity(nc, ident[:])

            # ---- embedding prep: emb = es / clip(cu, EPS); embT = emb.T ----
            with (
                tc.tile_pool(name="prep", bufs=3) as ppool,
                tc.tile_pool(name="prep_ps", bufs=2, space="PSUM") as pppool,
            ):
                for kt in range(NKT):
                    cu_t = ppool.tile([128, 1], F32, tag="cu")
                    nc.sync.dma_start(out=cu_t[:], in_=cu[kt * 128:(kt + 1) * 128, :])
                    inv = ppool.tile([128, 1], F32, tag="inv")
                    nc.vector.tensor_scalar_max(cu_t[:], cu_t[:], EPS)
                    nc.vector.reciprocal(inv[:], cu_t[:])
                    es_t = ppool.tile([128, SEM_DIM], F32, tag="es")
                    nc.sync.dma_start(out=es_t[:], in_=es[kt * 128:(kt + 1) * 128, :])
                    emb_t = ppool.tile([128, SEM_DIM], F32, tag="emb")
                    nc.vector.tensor_scalar_mul(emb_t[:], es_t[:], inv[:, 0:1])
                    for dc in range(4):
                        tp = pppool.tile([128, 128], F32, tag="tp")
                        nc.tensor.transpose(tp[:], emb_t[:, dc * 128:(dc + 1) * 128],
                                            ident[:])
                        nc.scalar.copy(
                            out=embT[dc][:, kt * 128:(kt + 1) * 128], in_=tp[:])

                # ---- negnorm[k] = -0.5 * sum_d embT[d,k]^2  (via ones matmul) ----
                npsums = [pppool.tile([1, 512], F32, tag=f"np{kc}",
                                      name=f"np{kc}", bufs=1) for kc in range(NKC)]
                for kc in range(NKC):
                    for dc in range(4):
                        sq = ppool.tile([128, 512], F32R, tag="sq")
                        sl = embT[dc][:, kc * 512:(kc + 1) * 512]
                        nc.gpsimd.tensor_mul(sq[:], sl, sl)
                        nc.tensor.matmul(npsums[kc][:], _r(ones_col[:]), _r(sq[:]),
                                         start=(dc == 0), stop=(dc == 3))
                    nc.scalar.activation(negnorm[0:1, kc * 512:(kc + 1) * 512],
                                         npsums[kc][:],
                                         mybir.ActivationFunctionType.Copy,
                                         scale=-0.5)

            # ---- main cdist loop (with interleaved acoustic chunks) ----
            iacc = cpool.tile([128, NT * 8], U32, tag="iacc")
            NCHUNK = 16
            W = T // NCHUNK

            with (
                tc.tile_pool(name="xp", bufs=8) as xpool,
                tc.tile_pool(name="sp", bufs=3) as spool,
                tc.tile_pool(name="mp", bufs=4) as mpool,
                tc.tile_pool(name="aco", bufs=2) as apool,
                tc.tile_pool(name="ps", bufs=8, space="PSUM") as pspool,
            ):
                msk = cpool.tile([BPC * ACO, 3], F32, tag="msk")
                nc.sync.dma_start(out=msk[:], in_=masks[:, :])

                def aco_chunk(ch):
                    csl = slice(ch * W, (ch + 1) * W)
                    xa_t = apool.tile([BPC * ACO, W], F32, tag="xa", name="xa_t")
                    nc.sync.dma_start(out=xa_t[:], in_=xa[:, csl])
                    zb = apool.tile([BPC * ACO, W], F32, tag="zb", name="zb")
                    nc.scalar.activation(zb[:], xa_t[:],
                                         mybir.ActivationFunctionType.Tanh)
                    nc.gpsimd.tensor_scalar_mul(zb[:], zb[:], HALF)
                    zq = apool.tile([BPC * ACO, W], F32, tag="zq", name="zq")
                    nc.gpsimd.tensor_scalar_add(zq[:], zb[:], RND)
                    nc.gpsimd.tensor_scalar_sub(zq[:], zq[:], RND)
                    ns = apool.tile([BPC * ACO, W], F32, tag="ns", name="ns")
                    nc.sync.dma_start(out=ns[:], in_=nz[:, csl])
                    nc.gpsimd.tensor_scalar(ns[:], ns[:], 2.0 / 9.0 * HALF,
                                            HALF / 9.0, mybir.AluOpType.mult,
                                            mybir.AluOpType.subtract)
                    zd = apool.tile([BPC * ACO, W], F32, tag="zd", name="zd")
                    nc.gpsimd.tensor_add(zd[:], zb[:], ns[:])
                    nc.gpsimd.tensor_scalar(zd[:], zd[:], -HALF, HALF,
                                            mybir.AluOpType.max,
                                            mybir.AluOpType.min)
                    zo = apool.tile([BPC * ACO, W], F32, tag="zo", name="zo")
                    nc.gpsimd.tensor_scalar_mul(zo[:], zq[:], msk[:, 0:1])
                    nc.gpsimd.tensor_scalar_mul(zd[:], zd[:], msk[:, 1:2])
                    nc.gpsimd.tensor_add(zo[:], zo[:], zd[:])
                    nc.gpsimd.tensor_scalar_mul(zb[:], zb[:], msk[:, 2:3])
                    nc.gpsimd.tensor_add(zo[:], zo[:], zb[:])
                    cf = apool.tile([BPC * ACO, W], F32, tag="cf", name="cf")
                    nc.gpsimd.tensor_scalar_add(cf[:], zo[:], HALF + RND)
                    nc.gpsimd.tensor_scalar_sub(cf[:], cf[:], RND)
                    ci = apool.tile([BPC * ACO, W], I32, tag="ci", name="ci")
                    nc.gpsimd.tensor_copy(ci[:], cf[:])
                    nc.sync.dma_start(out=c_aco[:, csl], in_=ci[:])
                    nc.gpsimd.tensor_scalar_mul(zo[:], zo[:], 1.0 / HALF)
                    nc.sync.dma_start(out=q_aco[:, csl], in_=zo[:])

                for tt in range(NT):
                    if tt % (NT // NCHUNK) == 2:
                        aco_chunk(tt // (NT // NCHUNK))
                    b = tt // TPB
                    t0 = (tt % TPB) * 128
                    xt = xpool.tile([128, SEM_DIM], F32R, tag="xt")
                    nc.sync.dma_start(
                        out=xt[:].rearrange("p (dc t) -> p dc t", t=128),
                        in_=xs[b, :, t0:t0 + 128].rearrange(
                            "(dc p) t -> p dc t", p=128))
                    xts = [xt[:, dc * 128:(dc + 1) * 128] for dc in range(4)]
                    s16 = spool.tile([128, KP], F16, tag="s16")
                    for kc in range(NKC):
                        ps = pspool.tile([128, 512], F32, tag="ps")
                        ksl = slice(kc * 512, (kc + 1) * 512)
                        nc.tensor.matmul(ps[:], _r(ones_row[:]),
                                         _r(negnorm[0:1, ksl]),
                                         start=True, stop=False)
                        for dc in range(4):
                            nc.tensor.matmul(ps[:], xts[dc],
                                             _r(embT[dc][:, ksl]),
                                             start=False, stop=(dc == 3))
                        nc.scalar.activation(s16[:, ksl], ps[:],
                                             mybir.ActivationFunctionType.Copy,
                                             scale=1.0 / 16.0)
                    m8 = mpool.tile([128, 8], F16, tag="m8")
                    nc.vector.max(out=m8[:], in_=s16[:])
                    nc.vector.max_index(out=iacc[:, tt * 8:(tt + 1) * 8],
                                        in_max=m8[:], in_values=s16[:])
                nc.sync.dma_start(out=top8i[:, :], in_=iacc[:])

    nc.compile()
    return nc


def build_neff_b():
    nc = bacc.Bacc("TRN2", target_bir_lowering=False, debug=False)

    xs = nc.dram_tensor("xs", [BPC, SEM_DIM, T], F32, kind="ExternalInput")
    embc = [
        nc.dram_tensor(f"embc{b}", [C, SEM_DIM], F32, kind="ExternalInput")
        for b in range(BPC)
    ]
    negnc = nc.dram_tensor("negnc", [BPC, C], F32, kind="ExternalInput")

    idxb = nc.dram_tensor("idxb", [128, NT * 8], U32, kind="ExternalOutput")
    q_sem = nc.dram_tensor("q_sem", [BPC, SEM_DIM, T], F32, kind="ExternalOutput")

    with tile.TileContext(nc) as tc:
        with tc.tile_pool(name="const", bufs=1) as cpool:
            ident = cpool.tile([128, 128], F32, tag="ident")
            make_identity(nc, ident[:])
            embCT = [[cpool.tile([128, C], F32, tag=f"eCT{b}_{dc}",
                                 name=f"eCT{b}_{dc}")
                      for dc in range(4)] for b in range(BPC)]
            bias = [cpool.tile([128, C], F32, tag=f"bias{b}",
                               name=f"bias{b}") for b in range(BPC)]

            with (
                tc.tile_pool(name="prep", bufs=3) as ppool,
                tc.tile_pool(name="prep_ps", bufs=2, space="PSUM") as pppool,
            ):
                for b in range(BPC):
                    nc.sync.dma_start(
                        out=bias[b][:],
                        in_=negnc[b:b + 1, :].to_broadcast([128, C]))
                    for kt in range(C // 128):
                        ec = ppool.tile([128, SEM_DIM], F32, tag="ec")
                        nc.sync.dma_start(
                            out=ec[:], in_=embc[b][kt * 128:(kt + 1) * 128, :])
                        for dc in range(4):
                            tp = pppool.tile([128, 128], F32, tag="tp")
                            nc.tensor.transpose(
                                tp[:], ec[:, dc * 128:(dc + 1) * 128], ident[:])
                            nc.scalar.copy(
                                out=embCT[b][dc][:, kt * 128:(kt + 1) * 128],
                                in_=tp[:])

            iacc = cpool.tile([128, NT * 8], U32, tag="iacc")
            with (
                tc.tile_pool(name="xp", bufs=3) as xpool,
                tc.tile_pool(name="sc", bufs=2) as scpool,
                tc.tile_pool(name="mp", bufs=2) as mpool,
                tc.tile_pool(name="gp", bufs=2) as gpool,
                tc.tile_pool(name="ps", bufs=4, space="PSUM") as pspool,
                tc.tile_pool(name="tps", bufs=4, space="PSUM") as tpspool,
            ):
                for tt in range(NT):
                    b = tt // TPB
                    t0 = (tt % TPB) * 128
                    xt = xpool.tile([128, SEM_DIM], F32, tag="xt")
                    nc.sync.dma_start(
                        out=xt[:].rearrange("p (dc t) -> p dc t", t=128),
                        in_=xs[b, :, t0:t0 + 128].rearrange(
                            "(dc p) t -> p dc t", p=128))
                    ps = pspool.tile([128, C], F32, tag="ps")
                    for dc in range(4):
                        nc.tensor.matmul(ps[:], xt[:, dc * 128:(dc + 1) * 128],
                                         embCT[b][dc][:],
                                         start=(dc == 0), stop=(dc == 3))
                    sc = scpool.tile([128, C], F32, tag="sc")
                    nc.vector.tensor_add(sc[:], ps[:], bias[b][:])
                    m8 = mpool.tile([128, 8], F32, tag="m8")
                    i8 = iacc[:, tt * 8:(tt + 1) * 8]
                    nc.vector.max(out=m8[:], in_=sc[:])
                    nc.vector.max_index(out=i8, in_max=m8[:], in_values=sc[:])
                    g = gpool.tile([128, SEM_DIM], F32, tag="g")
                    nc.gpsimd.indirect_dma_start(
                        out=g[:], out_offset=None, in_=embc[b][:],
                        in_offset=bass.IndirectOffsetOnAxis(
                            ap=iacc[:, tt * 8:tt * 8 + 1], axis=0))
                    gt = gpool.tile([128, SEM_DIM], F32, tag="gt")
                    for dc in range(4):
                        tp = tpspool.tile([128, 128], F32, tag="tp")
                        nc.tensor.transpose(
                            tp[:], g[:, dc * 128:(dc + 1) * 128], ident[:])
                        nc.scalar.copy(out=gt[:, dc * 128:(dc + 1) * 128], in_=tp[:])
                    nc.sync.dma_start(
                        out=q_sem[b, :, t0:t0 + 128].rearrange(
                            "(dc p) t -> p dc t", p=128),
                        in_=gt[:].rearrange("p (dc t) -> p dc t", t=128))
                nc.sync.dma_start(out=idxb[:, :], in_=iacc[:])

    nc.compile()
    return nc


def kernel(x, embedding_sum, cluster_usage, noise, probs_sem, probs_aco):
    x = np.ascontiguousarray(np.asarray(x, dtype=np.float32))
    es = np.ascontiguousarray(np.asarray(embedding_sum, dtype=np.float32))
    cu = np.asarray(cluster_usage, dtype=np.float32)
    noise = np.asarray(noise, dtype=np.float32)
    probs_sem = np.asarray(probs_sem, dtype=np.float32)
    probs_aco = np.asarray(probs_aco, dtype=np.float32)

    # ---------- host: provably-safe codebook pruning for pass A ----------
    # Code k can appear in some vector's true top-8 only if
    # U_k = X*r_k - r_k^2/2 >= L8 = 8th-largest of (-X*r_k - r_k^2/2),
    # where X = max_t ||x_t||. Everything in fp64 with slack.
    emb64h = es.astype(np.float64) / np.clip(
        cu.astype(np.float64), EPS, None)[:, None]
    r64 = np.linalg.norm(emb64h, axis=1)
    X = float(np.sqrt((x[:, :SEM_DIM, :].astype(np.float64) ** 2)
                      .sum(1)).max())
    U = X * r64 - 0.5 * r64 * r64
    L8 = np.sort(-X * r64 - 0.5 * r64 * r64)[-8]
    surv = np.nonzero(U >= L8 - 1.0)[0]
    KP = min(KCB, max(512, int(np.ceil(len(surv) / 512.0)) * 512))
    surv_p = np.concatenate([surv, np.full(KP - len(surv), surv[0])])[:KP]
    surv_p = surv_p.astype(np.int64)

    # ---------- pass A ----------
    nc_a = build_neff_a(KP)
    in_maps_a = []
    es_p = np.ascontiguousarray(es[surv_p])
    cu_p = np.ascontiguousarray(cu[surv_p].reshape(KP, 1))
    for c in range(NCORES):
        gb = slice(c * BPC, (c + 1) * BPC)
        xs = np.ascontiguousarray(x[gb, :SEM_DIM, :])
        xa = np.ascontiguousarray(x[gb, SEM_DIM:, :].reshape(BPC * ACO, T))
        nz = np.ascontiguousarray(noise[gb].reshape(BPC * ACO, T))
        masks = np.zeros((BPC * ACO, 3), dtype=np.float32)
        for b in range(BPC):
            p = probs_aco[c * BPC + b]
            col = 0 if p < 0.5 else (1 if p < 0.75 else 2)
            masks[b * ACO:(b + 1) * ACO, col] = 1.0
        in_maps_a.append({"xs": xs, "xa": xa, "es": es_p, "cu": cu_p,
                          "nz": nz, "masks": masks})
    res_a = run_bass_kernel_spmd(nc_a, in_maps_a, list(range(NCORES)),
                                 trace=_TRACE["on"], **_TRACE["kwargs"])
    _LAST["a"] = res_a
    _LAST["nc_a"] = nc_a

    # ---------- host: candidate unions + exact tables ----------
    emb64 = es.astype(np.float64) / np.clip(
        cu.astype(np.float64), EPS, None)[:, None]
    emb32 = emb64.astype(np.float32)
    nrm64 = -0.5 * (emb32.astype(np.float64) ** 2).sum(1)

    c_lists = np.zeros((B, C), dtype=np.int64)
    for gb in range(B):
        c, b = gb // BPC, gb % BPC
        # top8i layout: [128, NT*8] -> vector (tt, p) top-8 at [p, tt*8:(tt+1)*8]
        ti = res_a.results[c]["top8i"].reshape(128, NT, 8)[:, b * TPB:(b + 1) * TPB]
        ti = surv_p[ti.reshape(-1).astype(np.int64)]
        uniq, counts = np.unique(ti, return_counts=True)
        if len(uniq) > C:
            keep = np.sort(uniq[np.argsort(-counts)[:C]])
        else:
            keep = uniq
        cl = np.sort(keep.astype(np.int64))
        c_lists[gb, :len(cl)] = cl
        c_lists[gb, len(cl):] = cl[-1] if len(cl) else 0

    # ---------- pass B ----------
    nc_b = build_neff_b()
    in_maps_b = []
    for c in range(NCORES):
        gb = slice(c * BPC, (c + 1) * BPC)
        xs = np.ascontiguousarray(x[gb, :SEM_DIM, :])
        m = {"xs": xs}
        negnc = np.zeros((BPC, C), dtype=np.float32)
        for b in range(BPC):
            cl = c_lists[c * BPC + b]
            m[f"embc{b}"] = np.ascontiguousarray(emb32[cl])
            negnc[b] = nrm64[cl].astype(np.float32)
        m["negnc"] = negnc
        in_maps_b.append(m)
    res_b = run_bass_kernel_spmd(nc_b, in_maps_b, list(range(NCORES)),
                                 trace=_TRACE["on"], **_TRACE["kwargs"])
    _LAST["b"] = res_b
    _LAST["nc_b"] = nc_b

    # ---------- assemble ----------
    quantized = np.empty((B, SEM_DIM + ACO, T), dtype=np.float32)
    codes = np.empty((B, 1 + ACO, T), dtype=np.int32)
    for c in range(NCORES):
        ra, rb = res_a.results[c], res_b.results[c]
        for b in range(BPC):
            gb = c * BPC + b
            if probs_sem[gb] < 0.5:
                quantized[gb, :SEM_DIM] = rb["q_sem"][b]
            else:
                quantized[gb, :SEM_DIM] = x[gb, :SEM_DIM]
            quantized[gb, SEM_DIM:] = ra["q_aco"][b * ACO:(b + 1) * ACO]
            j = rb["idxb"].reshape(128, NT, 8)[:, b * TPB:(b + 1) * TPB, 0]
            j = j.T.reshape(T)
            codes[gb, 0] = c_lists[gb][j.astype(np.int64)].astype(np.int32)
            codes[gb, 1:] = ra["c_aco"][b * ACO:(b + 1) * ACO]
    return quantized, codes


# revision 20
# speedup vs baseline: 1.0222x; 1.0007x over previous
"""Trainium2 Bass kernel for nn_MistralAudioCodebook (VQ codebook).

Strategy (8 NeuronCores, data-parallel over batch; 2 examples/core):
  Pass A (NEFF-A): coarse full cdist scores S = x.e_k - 0.5||e_k||^2 via
    float32r matmuls (full-rate fp32-width PE mode, ~tf32 precision),
    per-vector top-8 via DVE InstMax/InstMaxIndex on fp16-scaled scores.
    Also computes the whole acoustic FSQ path.
  Host: per-example union of top-8 candidate codes (~210 of 2048), padded
    to 256; exact fp32 candidate table built on host.
  Pass B (NEFF-B): exact fp32 cdist against the 256 candidates, argmax,
    gather of winning embedding rows (indirect DMA) + PE transpose to
    d-major, producing quantized semantic rows and codes.

Empirically (and by construction) the true fp32 argmin is always within
the coarse top-2, so the top-8 union contains it with huge margin.
"""

import numpy as np

import concourse.bass as bass
import concourse.mybir as mybir
import concourse.tile as tile
from concourse import bacc
from concourse.bass_utils import run_bass_kernel_spmd
from concourse.masks import make_identity

F32 = mybir.dt.float32
F32R = mybir.dt.float32r
F16 = mybir.dt.float16
U32 = mybir.dt.uint32
I32 = mybir.dt.int32

SEM_DIM = 512
KCB = 2048          # codebook size
ACO = 32
B, T = 16, 4096
NCORES = 8
BPC = B // NCORES   # examples per core (2)
NT = BPC * T // 128  # t-tiles per core (64)
TPB = T // 128       # t-tiles per example (32)
C = 256              # candidate set size per example
EPS = 1e-5
HALF = 4.0
RND = 12582912.0     # 1.5 * 2^23 : RNE round-to-int magic constant

_TRACE = {"on": False, "kwargs": {}}
_LAST = {}


def _r(ap):
    return ap.bitcast(F32R)


def build_neff_a(KP=KCB):
    nc = bacc.Bacc("TRN2", target_bir_lowering=False, debug=False)
    NKT = KP // 128    # k-tiles in emb prep
    NKC = KP // 512    # 512-wide psum chunks

    xs = nc.dram_tensor("xs", [BPC, SEM_DIM, T], F32R, kind="ExternalInput")
    xa = nc.dram_tensor("xa", [BPC * ACO, T], F32, kind="ExternalInput")
    es = nc.dram_tensor("es", [KP, SEM_DIM], F32, kind="ExternalInput")
    cu = nc.dram_tensor("cu", [KP, 1], F32, kind="ExternalInput")
    nz = nc.dram_tensor("nz", [BPC * ACO, T], F32, kind="ExternalInput")
    masks = nc.dram_tensor("masks", [BPC * ACO, 3], F32, kind="ExternalInput")

    top8i = nc.dram_tensor("top8i", [128, NT * 8], U32, kind="ExternalOutput")
    q_aco = nc.dram_tensor("q_aco", [BPC * ACO, T], F32, kind="ExternalOutput")
    c_aco = nc.dram_tensor("c_aco", [BPC * ACO, T], I32, kind="ExternalOutput")

    with tile.TileContext(nc) as tc:
        with tc.tile_pool(name="const", bufs=1) as cpool:
            # Persistent tensors
            embT = [cpool.tile([128, KP], F32R, tag=f"embT{dc}",
                               name=f"embT{dc}") for dc in range(4)]
            negnorm = cpool.tile([1, KP], F32R, tag="negnorm")
            ones_col = cpool.tile([128, 1], F32R, tag="ones_col")   # lhsT for col-sums
            ones_row = cpool.tile([1, 128], F32R, tag="ones_row")   # lhsT for bias bcast
            ident = cpool.tile([128, 128], F32, tag="ident")
            ones_f = cpool.tile([128, 1], F32, tag="ones_f")
            onesr_f = cpool.tile([1, 128], F32, tag="onesr_f")
            nc.vector.memset(ones_f[:], 1.0)
            nc.vector.memset(onesr_f[:], 1.0)
            nc.scalar.copy(out=ones_col[:], in_=ones_f[:])
            nc.scalar.copy(out=ones_row[:], in_=onesr_f[:])
            make_identity(nc, ident[:])

            # ---- embedding prep: emb = es / clip(cu, EPS); embT = emb.T ----
            with (
                tc.tile_pool(name="prep", bufs=3) as ppool,
                tc.tile_pool(name="prep_ps", bufs=2, space="PSUM") as pppool,
            ):
                for kt in range(NKT):
                    cu_t = ppool.tile([128, 1], F32, tag="cu")
                    nc.sync.dma_start(out=cu_t[:], in_=cu[kt * 128:(kt + 1) * 128, :])
                    inv = ppool.tile([128, 1], F32, tag="inv")
                    nc.vector.tensor_scalar_max(cu_t[:], cu_t[:], EPS)
                    nc.vector.reciprocal(inv[:], cu_t[:])
                    es_t = ppool.tile([128, SEM_DIM], F32, tag="es")
                    nc.sync.dma_start(out=es_t[:], in_=es[kt * 128:(kt + 1) * 128, :])
                    emb_t = ppool.tile([128, SEM_DIM], F32, tag="emb")
                    nc.vector.tensor_scalar_mul(emb_t[:], es_t[:], inv[:, 0:1])
                    for dc in range(4):
                        tp = pppool.tile([128, 128], F32, tag="tp")
                        nc.tensor.transpose(tp[:], emb_t[:, dc * 128:(dc + 1) * 128],
                                            ident[:])
                        nc.scalar.copy(
                            out=embT[dc][:, kt * 128:(kt + 1) * 128], in_=tp[:])

                # ---- negnorm[k] = -0.5 * sum_d embT[d,k]^2  (via ones matmul) ----
                npsums = [pppool.tile([1, 512], F32, tag=f"np{kc}",
                                      name=f"np{kc}", bufs=1) for kc in range(NKC)]
                for kc in range(NKC):
                    for dc in range(4):
                        sq = ppool.tile([128, 512], F32R, tag="sq")
                        sl = embT[dc][:, kc * 512:(kc + 1) * 512]
                        nc.gpsimd.tensor_mul(sq[:], sl, sl)
                        nc.tensor.matmul(npsums[kc][:], _r(ones_col[:]), _r(sq[:]),
                                         start=(dc == 0), stop=(dc == 3))
                    nc.scalar.activation(negnorm[0:1, kc * 512:(kc + 1) * 512],
                                         npsums[kc][:],
                                         mybir.ActivationFunctionType.Copy,
                                         scale=-0.5)

            # ---- main cdist loop (with interleaved acoustic chunks) ----
            iacc = cpool.tile([128, NT * 8], U32, tag="iacc")
            NCHUNK = 16
            W = T // NCHUNK

            with (
                tc.tile_pool(name="xp", bufs=8) as xpool,
                tc.tile_pool(name="sp", bufs=3) as spool,
                tc.tile_pool(name="mp", bufs=4) as mpool,
                tc.tile_pool(name="aco", bufs=2) as apool,
                tc.tile_pool(name="ps", bufs=8, space="PSUM") as pspool,
            ):
                msk = cpool.tile([BPC * ACO, 3], F32, tag="msk")
                nc.sync.dma_start(out=msk[:], in_=masks[:, :])

                def aco_chunk(ch):
                    csl = slice(ch * W, (ch + 1) * W)
                    xa_t = apool.tile([BPC * ACO, W], F32, tag="xa", name="xa_t")
                    nc.sync.dma_start(out=xa_t[:], in_=xa[:, csl])
                    zb = apool.tile([BPC * ACO, W], F32, tag="zb", name="zb")
                    nc.scalar.activation(zb[:], xa_t[:],
                                         mybir.ActivationFunctionType.Tanh)
                    nc.gpsimd.tensor_scalar_mul(zb[:], zb[:], HALF)
                    zq = apool.tile([BPC * ACO, W], F32, tag="zq", name="zq")
                    nc.gpsimd.tensor_scalar_add(zq[:], zb[:], RND)
                    nc.gpsimd.tensor_scalar_sub(zq[:], zq[:], RND)
                    ns = apool.tile([BPC * ACO, W], F32, tag="ns", name="ns")
                    nc.sync.dma_start(out=ns[:], in_=nz[:, csl])
                    nc.gpsimd.tensor_scalar(ns[:], ns[:], 2.0 / 9.0 * HALF,
                                            HALF / 9.0, mybir.AluOpType.mult,
                                            mybir.AluOpType.subtract)
                    zd = apool.tile([BPC * ACO, W], F32, tag="zd", name="zd")
                    nc.gpsimd.tensor_add(zd[:], zb[:], ns[:])
                    nc.gpsimd.tensor_scalar(zd[:], zd[:], -HALF, HALF,
                                            mybir.AluOpType.max,
                                            mybir.AluOpType.min)
                    zo = apool.tile([BPC * ACO, W], F32, tag="zo", name="zo")
                    nc.gpsimd.tensor_scalar_mul(zo[:], zq[:], msk[:, 0:1])
                    nc.gpsimd.tensor_scalar_mul(zd[:], zd[:], msk[:, 1:2])
                    nc.gpsimd.tensor_add(zo[:], zo[:], zd[:])
                    nc.gpsimd.tensor_scalar_mul(zb[:], zb[:], msk[:, 2:3])
                    nc.gpsimd.tensor_add(zo[:], zo[:], zb[:])
                    cf = apool.tile([BPC * ACO, W], F32, tag="cf", name="cf")
                    nc.gpsimd.tensor_scalar_add(cf[:], zo[:], HALF + RND)
                    nc.gpsimd.tensor_scalar_sub(cf[:], cf[:], RND)
                    ci = apool.tile([BPC * ACO, W], I32, tag="ci", name="ci")
                    nc.gpsimd.tensor_copy(ci[:], cf[:])
                    nc.sync.dma_start(out=c_aco[:, csl], in_=ci[:])
                    nc.gpsimd.tensor_scalar_mul(zo[:], zo[:], 1.0 / HALF)
                    nc.sync.dma_start(out=q_aco[:, csl], in_=zo[:])

                for tt in range(NT):
                    if tt % (NT // NCHUNK) == 2:
                        aco_chunk(tt // (NT // NCHUNK))
                    b = tt // TPB
                    t0 = (tt % TPB) * 128
                    xt = xpool.tile([128, SEM_DIM], F32R, tag="xt")
                    nc.sync.dma_start(
                        out=xt[:].rearrange("p (dc t) -> p dc t", t=128),
                        in_=xs[b, :, t0:t0 + 128].rearrange(
                            "(dc p) t -> p dc t", p=128))
                    xts = [xt[:, dc * 128:(dc + 1) * 128] for dc in range(4)]
                    s16 = spool.tile([128, KP], F16, tag="s16")
                    for kc in range(NKC):
                        ps = pspool.tile([128, 512], F32, tag="ps")
                        ksl = slice(kc * 512, (kc + 1) * 512)
                        nc.tensor.matmul(ps[:], _r(ones_row[:]),
                                         _r(negnorm[0:1, ksl]),
                                         start=True, stop=False)
                        for dc in range(4):
                            nc.tensor.matmul(ps[:], xts[dc],
                                             _r(embT[dc][:, ksl]),
                                             start=False, stop=(dc == 3))
                        nc.scalar.activation(s16[:, ksl], ps[:],
                                             mybir.ActivationFunctionType.Copy,
                                             scale=1.0 / 16.0)
                    m8 = mpool.tile([128, 8], F16, tag="m8")
                    nc.vector.max(out=m8[:], in_=s16[:])
                    nc.vector.max_index(out=iacc[:, tt * 8:(tt + 1) * 8],
                                        in_max=m8[:], in_values=s16[:])
                    if tt == NT // 2 - 1:
                        nc.sync.dma_start(out=top8i[:, :NT * 4],
                                          in_=iacc[:, :NT * 4])
                nc.sync.dma_start(out=top8i[:, NT * 4:], in_=iacc[:, NT * 4:])

    nc.compile()
    return nc


def build_neff_b():
    nc = bacc.Bacc("TRN2", target_bir_lowering=False, debug=False)

    xs = nc.dram_tensor("xs", [BPC, SEM_DIM, T], F32, kind="ExternalInput")
    embc = [
        nc.dram_tensor(f"embc{b}", [C, SEM_DIM], F32, kind="ExternalInput")
        for b in range(BPC)
    ]
    negnc = nc.dram_tensor("negnc", [BPC, C], F32, kind="ExternalInput")

    idxb = nc.dram_tensor("idxb", [128, NT * 8], U32, kind="ExternalOutput")
    q_sem = nc.dram_tensor("q_sem", [BPC, SEM_DIM, T], F32, kind="ExternalOutput")

    with tile.TileContext(nc) as tc:
        with tc.tile_pool(name="const", bufs=1) as cpool:
            ident = cpool.tile([128, 128], F32, tag="ident")
            make_identity(nc, ident[:])
            embCT = [[cpool.tile([128, C], F32, tag=f"eCT{b}_{dc}",
                                 name=f"eCT{b}_{dc}")
                      for dc in range(4)] for b in range(BPC)]
            bias = [cpool.tile([128, C], F32, tag=f"bias{b}",
                               name=f"bias{b}") for b in range(BPC)]

            with (
                tc.tile_pool(name="prep", bufs=3) as ppool,
                tc.tile_pool(name="prep_ps", bufs=2, space="PSUM") as pppool,
            ):
                for b in range(BPC):
                    nc.sync.dma_start(
                        out=bias[b][:],
                        in_=negnc[b:b + 1, :].to_broadcast([128, C]))
                    for kt in range(C // 128):
                        ec = ppool.tile([128, SEM_DIM], F32, tag="ec")
                        nc.sync.dma_start(
                            out=ec[:], in_=embc[b][kt * 128:(kt + 1) * 128, :])
                        for dc in range(4):
                            tp = pppool.tile([128, 128], F32, tag="tp")
                            nc.tensor.transpose(
                                tp[:], ec[:, dc * 128:(dc + 1) * 128], ident[:])
                            nc.scalar.copy(
                                out=embCT[b][dc][:, kt * 128:(kt + 1) * 128],
                                in_=tp[:])

            iacc = cpool.tile([128, NT * 8], U32, tag="iacc")
            with (
                tc.tile_pool(name="xp", bufs=3) as xpool,
                tc.tile_pool(name="sc", bufs=2) as scpool,
                tc.tile_pool(name="mp", bufs=2) as mpool,
                tc.tile_pool(name="gp", bufs=2) as gpool,
                tc.tile_pool(name="ps", bufs=4, space="PSUM") as pspool,
                tc.tile_pool(name="tps", bufs=4, space="PSUM") as tpspool,
            ):
                for tt in range(NT):
                    b = tt // TPB
                    t0 = (tt % TPB) * 128
                    xt = xpool.tile([128, SEM_DIM], F32, tag="xt")
                    nc.sync.dma_start(
                        out=xt[:].rearrange("p (dc t) -> p dc t", t=128),
                        in_=xs[b, :, t0:t0 + 128].rearrange(
                            "(dc p) t -> p dc t", p=128))
                    ps = pspool.tile([128, C], F32, tag="ps")
                    for dc in range(4):
                        nc.tensor.matmul(ps[:], xt[:, dc * 128:(dc + 1) * 128],
                                         embCT[b][dc][:],
                                         start=(dc == 0), stop=(dc == 3))
                    sc = scpool.tile([128, C], F32, tag="sc")
                    nc.vector.tensor_add(sc[:], ps[:], bias[b][:])
                    m8 = mpool.tile([128, 8], F32, tag="m8")
                    i8 = iacc[:, tt * 8:(tt + 1) * 8]
                    nc.vector.max(out=m8[:], in_=sc[:])
                    nc.vector.max_index(out=i8, in_max=m8[:], in_values=sc[:])
                    g = gpool.tile([128, SEM_DIM], F32, tag="g")
                    nc.gpsimd.indirect_dma_start(
                        out=g[:], out_offset=None, in_=embc[b][:],
                        in_offset=bass.IndirectOffsetOnAxis(
                            ap=iacc[:, tt * 8:tt * 8 + 1], axis=0))
                    gt = gpool.tile([128, SEM_DIM], F32, tag="gt")
                    for dc in range(4):
                        tp = tpspool.tile([128, 128], F32, tag="tp")
                        nc.tensor.transpose(
                            tp[:], g[:, dc * 128:(dc + 1) * 128], ident[:])
                        nc.scalar.copy(out=gt[:, dc * 128:(dc + 1) * 128], in_=tp[:])
                    nc.sync.dma_start(
                        out=q_sem[b, :, t0:t0 + 128].rearrange(
                            "(dc p) t -> p dc t", p=128),
                        in_=gt[:].rearrange("p (dc t) -> p dc t", t=128))
                nc.sync.dma_start(out=idxb[:, :], in_=iacc[:])

    nc.compile()
    return nc


def kernel(x, embedding_sum, cluster_usage, noise, probs_sem, probs_aco):
    x = np.ascontiguousarray(np.asarray(x, dtype=np.float32))
    es = np.ascontiguousarray(np.asarray(embedding_sum, dtype=np.float32))
    cu = np.asarray(cluster_usage, dtype=np.float32)
    noise = np.asarray(noise, dtype=np.float32)
    probs_sem = np.asarray(probs_sem, dtype=np.float32)
    probs_aco = np.asarray(probs_aco, dtype=np.float32)

    # ---------- host: provably-safe codebook pruning for pass A ----------
    # Code k can appear in some vector's true top-8 only if
    # U_k = X*r_k - r_k^2/2 >= L8 = 8th-largest of (-X*r_k - r_k^2/2),
    # where X = max_t ||x_t||. Everything in fp64 with slack.
    emb64h = es.astype(np.float64) / np.clip(
        cu.astype(np.float64), EPS, None)[:, None]
    r64 = np.linalg.norm(emb64h, axis=1)
    X = float(np.sqrt((x[:, :SEM_DIM, :].astype(np.float64) ** 2)
                      .sum(1)).max())
    U = X * r64 - 0.5 * r64 * r64
    L8 = np.sort(-X * r64 - 0.5 * r64 * r64)[-8]
    surv = np.nonzero(U >= L8 - 1.0)[0]
    KP = min(KCB, max(512, int(np.ceil(len(surv) / 512.0)) * 512))
    surv_p = np.concatenate([surv, np.full(KP - len(surv), surv[0])])[:KP]
    surv_p = surv_p.astype(np.int64)

    # ---------- pass A ----------
    nc_a = build_neff_a(KP)
    in_maps_a = []
    es_p = np.ascontiguousarray(es[surv_p])
    cu_p = np.ascontiguousarray(cu[surv_p].reshape(KP, 1))
    for c in range(NCORES):
        gb = slice(c * BPC, (c + 1) * BPC)
        xs = np.ascontiguousarray(x[gb, :SEM_DIM, :])
        xa = np.ascontiguousarray(x[gb, SEM_DIM:, :].reshape(BPC * ACO, T))
        nz = np.ascontiguousarray(noise[gb].reshape(BPC * ACO, T))
        masks = np.zeros((BPC * ACO, 3), dtype=np.float32)
        for b in range(BPC):
            p = probs_aco[c * BPC + b]
            col = 0 if p < 0.5 else (1 if p < 0.75 else 2)
            masks[b * ACO:(b + 1) * ACO, col] = 1.0
        in_maps_a.append({"xs": xs, "xa": xa, "es": es_p, "cu": cu_p,
                          "nz": nz, "masks": masks})
    res_a = run_bass_kernel_spmd(nc_a, in_maps_a, list(range(NCORES)),
                                 trace=_TRACE["on"], **_TRACE["kwargs"])
    _LAST["a"] = res_a
    _LAST["nc_a"] = nc_a

    # ---------- host: candidate unions + exact tables ----------
    emb64 = es.astype(np.float64) / np.clip(
        cu.astype(np.float64), EPS, None)[:, None]
    emb32 = emb64.astype(np.float32)
    nrm64 = -0.5 * (emb32.astype(np.float64) ** 2).sum(1)

    c_lists = np.zeros((B, C), dtype=np.int64)
    for gb in range(B):
        c, b = gb // BPC, gb % BPC
        # top8i layout: [128, NT*8] -> vector (tt, p) top-8 at [p, tt*8:(tt+1)*8]
        ti = res_a.results[c]["top8i"].reshape(128, NT, 8)[:, b * TPB:(b + 1) * TPB]
        ti = surv_p[ti.reshape(-1).astype(np.int64)]
        uniq, counts = np.unique(ti, return_counts=True)
        if len(uniq) > C:
            keep = np.sort(uniq[np.argsort(-counts)[:C]])
        else:
            keep = uniq
        cl = np.sort(keep.astype(np.int64))
        c_lists[gb, :len(cl)] = cl
        c_lists[gb, len(cl):] = cl[-1] if len(cl) else 0

    # ---------- pass B ----------
    nc_b = build_neff_b()
    in_maps_b = []
    for c in range(NCORES):
        gb = slice(c * BPC, (c + 1) * BPC)
        xs = np.ascontiguousarray(x[gb, :SEM_DIM, :])
        m = {"xs": xs}
        negnc = np.zeros((BPC, C), dtype=np.float32)
        for b in range(BPC):
            cl = c_lists[c * BPC + b]
            m[f"embc{b}"] = np.ascontiguousarray(emb32[cl])
            negnc[b] = nrm64[cl].astype(np.float32)
        m["negnc"] = negnc
        in_maps_b.append(m)
    res_b = run_bass_kernel_spmd(nc_b, in_maps_b, list(range(NCORES)),
                                 trace=_TRACE["on"], **_TRACE["kwargs"])
    _LAST["b"] = res_b
    _LAST["nc_b"] = nc_b

    # ---------- assemble ----------
    quantized = np.empty((B, SEM_DIM + ACO, T), dtype=np.float32)
    codes = np.empty((B, 1 + ACO, T), dtype=np.int32)
    for c in range(NCORES):
        ra, rb = res_a.results[c], res_b.results[c]
        for b in range(BPC):
            gb = c * BPC + b
            if probs_sem[gb] < 0.5:
                quantized[gb, :SEM_DIM] = rb["q_sem"][b]
            else:
                quantized[gb, :SEM_DIM] = x[gb, :SEM_DIM]
            quantized[gb, SEM_DIM:] = ra["q_aco"][b * ACO:(b + 1) * ACO]
            j = rb["idxb"].reshape(128, NT, 8)[:, b * TPB:(b + 1) * TPB, 0]
            j = j.T.reshape(T)
            codes[gb, 0] = c_lists[gb][j.astype(np.int64)].astype(np.int32)
            codes[gb, 1:] = ra["c_aco"][b * ACO:(b + 1) * ACO]
    return quantized, codes
